# revision 19
# baseline (speedup 1.0000x reference)
"""CSWin strip-window attention + pooling kernel for Trainium2 (8 NeuronCores).

Problem: nn_CswinPool — fmap (16, 256, 64, 64) f32.
  qv_h = conv(fmap[:, :128], W_qvh, stride (2,1));  qv_v = conv(fmap[:, 128:], W_qvv, stride (1,2))
  k_all = W_k @ fmap; lepe_all = W_lepe @ fmap (1x1 convs)
  two strip-window attention branches (horizontal / vertical), 8 heads, d=16
  out = fmap + W_proj @ (attn_out + lepe_all)

Strategy (data-parallel over batch, 2 images per core):
  - lepe folded host-side: out = W_r @ F + W_projH' @ AV_H + W_projV' @ AV_V,
    W_r = I + W_proj @ W_lepe  (identity residual folded in)
  - channel permutation sigma puts head h at partitions 32*(h//2)+16*(h%2)
    so row/col-tiled PE matmuls hit quadrant-aligned SBUF bases
  - dense 1x1/strided convs as fp32r matmuls (full PE rate, no conversion pass)
  - QK row-tiled 4 concurrent heads (even heads K=16; odd heads K=32 with a
    zero-padded q buffer produced by an extra zeroed-weight conv output)
  - softmax without max-subtraction (|S| < 0.6 for this problem), exp on ACT
    in one (128, 2048) instruction per group, row-sums via GPSIMD+DVE tree
  - AV col-tiled, P moving, v'(= v/r) stationary; output (h,d)-contiguous in
    PSUM so evac is full-128-partition
"""
import sys
import os

for _p in ("/opt/trn_rl_repo", "/root/.axon_site/_ro/trn_rl_repo"):
    if _p not in sys.path and os.path.isdir(_p):
        sys.path.insert(0, _p)

import numpy as np
import ml_dtypes
from contextlib import ExitStack

import concourse.bass as bass
import concourse.tile as tile
from concourse import bacc, mybir
from concourse.bass_utils import run_bass_kernel_spmd
from concourse.masks import make_identity

F32 = mybir.dt.float32
F32R = mybir.dt.float32r
BF16 = mybir.dt.bfloat16

N_CORES = 8
B_PER_CORE = 2
DIM = 256
HEADS = 8
D = 16           # per-branch head dim
X = Y = 64
S_SPATIAL = X * Y
SCALE = (DIM // HEADS) ** -0.5

# partition p holds reference channel SIGMA[p]
SIGMA = np.zeros(128, dtype=np.int64)
for _h in range(8):
    for _d in range(16):
        SIGMA[32 * (_h // 2) + 16 * (_h % 2) + _d] = _h * 16 + _d


def prep_weights(W_qvh, W_qvv, W_k, W_lepe, W_proj):
    """Host-side weight preparation: permutations, folds, lhsT layouts."""
    W_qvh = np.asarray(W_qvh, dtype=np.float32)
    W_qvv = np.asarray(W_qvv, dtype=np.float32)
    W_k = np.asarray(W_k, dtype=np.float32)
    W_lepe = np.asarray(W_lepe, dtype=np.float32)
    W_proj = np.asarray(W_proj, dtype=np.float32)

    # wk[half, cc] : (128 c, 128 o) lhsT for k matmul, output channels sigma-permuted
    wk = np.zeros((2, 2, 128, 128), dtype=np.float32)
    for half in range(2):
        Wh = W_k[128 * half:128 * half + 128][SIGMA]      # (128 o_perm, 256 c)
        for cc in range(2):
            wk[half, cc] = Wh[:, 128 * cc:128 * cc + 128].T

    # wcv[branch, kind(q=0,v=1,qz=2), tap] : (128 c, 128 o)
    wcv = np.zeros((2, 3, 2, 128, 128), dtype=np.float32)
    zero_even = np.ones(128, dtype=np.float32)
    zero_even[(np.arange(128) % 32) < 16] = 0.0            # keep only odd-slot channels
    for br, Wc in ((0, W_qvh[:, :, :, 0]), (1, W_qvv[:, :, 0, :])):
        # Wc: (256 o, 128 c, 2 tap)
        Wq = Wc[:128][SIGMA] * SCALE                       # (128 o_perm, 128 c, 2)
        Wv = Wc[128:][SIGMA]
        Wqz = Wq * zero_even[:, None, None]
        for t in range(2):
            wcv[br, 0, t] = Wq[:, :, t].T
            wcv[br, 1, t] = Wv[:, :, t].T
            wcv[br, 2, t] = Wqz[:, :, t].T

    # wr[cc, oc] : (128 c, 128 o) lhsT of W_r = I + W_proj @ W_lepe
    W_r = np.eye(256, dtype=np.float32) + W_proj @ W_lepe
    wr = np.zeros((2, 2, 128, 128), dtype=np.float32)
    for cc in range(2):
        for oc in range(2):
            wr[cc, oc] = W_r[128 * oc:128 * oc + 128, 128 * cc:128 * cc + 128].T

    # wproj[branch, oc] : (128 c', 128 o) bf16, c' in sigma layout
    wproj = np.zeros((2, 2, 128, 128), dtype=ml_dtypes.bfloat16)
    for br in range(2):
        Wp = W_proj[:, 128 * br:128 * br + 128][:, SIGMA]  # (256 o, 128 c'_perm)
        for oc in range(2):
            wproj[br, oc] = Wp[128 * oc:128 * oc + 128, :].T.astype(ml_dtypes.bfloat16)

    wall = np.zeros((128, 20 * 128), dtype=np.float32)
    i = 0
    for half in range(2):
        for cc in range(2):
            wall[:, 128 * i:128 * i + 128] = wk[half, cc]
            i += 1
    for br in range(2):
        for kind in range(3):
            for t in range(2):
                wall[:, 128 * i:128 * i + 128] = wcv[br, kind, t]
                i += 1
    for cc in range(2):
        for oc in range(2):
            wall[:, 128 * i:128 * i + 128] = wr[cc, oc]
            i += 1
    wpall = np.zeros((128, 2 * 2 * 128), dtype=ml_dtypes.bfloat16)
    i = 0
    for br in range(2):
        for oc in range(2):
            wpall[:, 128 * i:128 * i + 128] = wproj[br, oc]
            i += 1
    return {"wall": wall, "wpall": wpall}


def r32(ap):
    return ap.bitcast(F32R)


def emit_kernel(tc, fmap_d, out_d, wall_d, wpall_d):
    nc = tc.nc
    with ExitStack() as ctx:
        singles = ctx.enter_context(tc.tile_pool(name="singles", bufs=1))
        fpool = ctx.enter_context(tc.tile_pool(name="fpool", bufs=2))
        kpool = ctx.enter_context(tc.tile_pool(name="kpool", bufs=1))
        kpool2 = ctx.enter_context(tc.tile_pool(name="kpool2", bufs=2))
        qvpool = ctx.enter_context(tc.tile_pool(name="qvpool", bufs=1))
        qvpool2 = ctx.enter_context(tc.tile_pool(name="qvpool2", bufs=2))
        avpool = ctx.enter_context(tc.tile_pool(name="avpool", bufs=2))
        avpool1 = ctx.enter_context(tc.tile_pool(name="avpool1", bufs=1))
        ppool = ctx.enter_context(tc.tile_pool(name="ppool", bufs=3))
        small = ctx.enter_context(tc.tile_pool(name="small", bufs=4))
        dense_ps = ctx.enter_context(tc.tile_pool(name="dense_ps", bufs=2, space="PSUM"))
        s_ps = ctx.enter_context(tc.tile_pool(name="s_ps", bufs=1, space="PSUM"))
        vt_ps = ctx.enter_context(tc.tile_pool(name="vt_ps", bufs=1, space="PSUM"))
        av_ps = ctx.enter_context(tc.tile_pool(name="av_ps", bufs=1, space="PSUM"))

        # ---- weights (two big DMAs; slices as lhsT views) ----
        wall_sb = singles.tile([128, 2560], F32R, tag="wall", name="wall")
        nc.sync.dma_start(wall_sb, wall_d)
        wpall_sb = singles.tile([128, 512], BF16, tag="wpall", name="wpall")
        nc.sync.dma_start(wpall_sb, wpall_d)

        def wslice(i):
            return wall_sb[:, 128 * i:128 * i + 128]

        wk_sb = [[wslice(h * 2 + c) for c in range(2)] for h in range(2)]
        wcv_sb = [[[wslice(4 + b * 6 + k * 2 + t) for t in range(2)]
                   for k in range(3)] for b in range(2)]
        wr_sb = [[wslice(16 + c * 2 + o) for o in range(2)] for c in range(2)]
        wproj_sb = [[wpall_sb[:, 128 * (b * 2 + o):128 * (b * 2 + o) + 128] for o in range(2)] for b in range(2)]
        ident = singles.tile([128, 128], BF16, tag="ident", name="ident")
        make_identity(nc, ident)

        def dense_gen(img, st, h_first=False):
            """Yield after each F-chunk DMA / dense-conv chunk emission.

            Order matters: items whose destination buffers are free
            (double-buffered) come first so interleaved emission into the
            previous image's attention doesn't stall the pipeline."""
            F0 = fpool.tile([128, 4096], F32R, tag="F0", name="F0")
            F1 = fpool.tile([128, 4096], F32R, tag="F1", name="F1")
            st["F"] = (F0, F1)
            fm = [fmap_d[img, 0:128].rearrange("c x y -> c (x y)"),
                  fmap_d[img, 128:256].rearrange("c x y -> c (x y)")]
            for half, Ft in ((0, F0), (1, F1)):
                for n in range(8):
                    nc.sync.dma_start(Ft[:, 512 * n:512 * n + 512],
                                      fm[half][:, 512 * n:512 * n + 512])
                    yield
            Fs = (F0, F1)

            def k_chunks(half, ksb):
                for n in range(8):
                    pt = dense_ps.tile([128, 512], F32, tag="dps", name="dps")
                    for cc in range(2):
                        nc.tensor.matmul(out=pt, lhsT=wk_sb[half][cc],
                                         rhs=Fs[cc][:, 512 * n:512 * n + 512],
                                         start=(cc == 0), stop=(cc == 1))
                    if n % 4 == 3:
                        nc.scalar.activation(out=ksb[:, 512 * n:512 * n + 512], in_=pt,
                                             func=mybir.ActivationFunctionType.Copy)
                    else:
                        nc.vector.tensor_copy(out=ksb[:, 512 * n:512 * n + 512], in_=pt)
                    yield

            def qv_chunks(br, kind, dst, reorder):
                Fv = (Fs[0].rearrange("c (i t y) -> c i t y", t=2, y=64) if br == 0
                      else Fs[1].rearrange("c (x j t) -> c x j t", j=32, t=2))
                dstv = dst.rearrange("c (g i p) -> c i g p", g=16, p=4) if reorder else dst
                for n in range(4):
                    pt = dense_ps.tile([128, 512], F32, tag="dps", name="dps")
                    for t in range(2):
                        rhs = (Fv[:, 8 * n:8 * n + 8, t, :] if br == 0
                               else Fv[:, 16 * n:16 * n + 16, :, t])
                        nc.tensor.matmul(out=pt, lhsT=wcv_sb[br][kind][t], rhs=rhs,
                                         start=(t == 0), stop=(t == 1))
                    if reorder:
                        nc.vector.tensor_copy(out=dstv[:, 8 * n:8 * n + 8], in_=pt)
                    else:
                        nc.vector.tensor_copy(out=dst[:, 512 * n:512 * n + 512], in_=pt)
                    yield

            def emit_V():
                kV = kpool2.tile([128, 4096], BF16, tag="kV", name="kV")
                st["kV"] = kV
                yield from k_chunks(1, kV)
                qV = qvpool2.tile([128, 2048], BF16, tag="qV", name="qV")
                st["qV"] = qV
                yield from qv_chunks(1, 0, qV, False)
                vV = qvpool2.tile([128, 2048], BF16, tag="vV", name="vV")
                st["vV"] = vV
                yield from qv_chunks(1, 1, vV, False)

            def emit_H():
                kH = kpool.tile([128, 4096], BF16, tag="kH", name="kH")
                st["kH"] = kH
                yield from k_chunks(0, kH)
                qH = qvpool.tile([128, 2048], BF16, tag="qH", name="qH")
                st["qH"] = qH
                yield from qv_chunks(0, 0, qH, True)
                vH = qvpool.tile([128, 2048], BF16, tag="vH", name="vH")
                st["vH"] = vH
                yield from qv_chunks(0, 1, vH, True)
                qzH = qvpool2.tile([128, 2048], BF16, tag="qz", name="qz", bufs=3)
                st["qzH"] = qzH
                yield from qv_chunks(0, 2, qzH, True)

            # for image 0 (nothing to overlap) emit H-branch buffers first so
            # attention starts ASAP; for interleaved images emit the
            # double-buffered (no-WAR-wait) V-branch destinations first.
            if h_first:
                yield from emit_H()
                yield from emit_V()
            else:
                yield from emit_V()
                yield from emit_H()
            qzV = qvpool2.tile([128, 2048], BF16, tag="qz", name="qz", bufs=3)
            st["qzV"] = qzV
            yield from qv_chunks(1, 2, qzV, False)

        def attention_group(st, br, g):
            ksb = st["kH"] if br == 0 else st["kV"]
            qsb = st["qH"] if br == 0 else st["qV"]
            vsb = st["vH"] if br == 0 else st["vV"]
            qzsb = st["qzH"] if br == 0 else st["qzV"]
            avsb = st["AVH"] if br == 0 else st["AVV"]
            kv = ksb.rearrange("c (x y) -> c x y", y=64)
            avv = avsb.rearrange("c (x y) -> c x y", y=64)
            if br == 0:
                k_ap = kv[:, :, 4 * g:4 * g + 4]                # (128, 64, 4)
                av_out = avv[:, :, 4 * g:4 * g + 4]
            else:
                k_ap = kv[:, 4 * g:4 * g + 4, :]                # (128, 4, 64)
                av_out = avv[:, 4 * g:4 * g + 4, :]
            qg = qsb[:, 128 * g:128 * g + 128]                  # grouped tokens
            qzg = qzsb[:, 128 * g:128 * g + 128]
            vg = vsb[:, 128 * g:128 * g + 128]

            # QK full group: S (128,2048), row-tile r owns bank r (bank-safe);
            # head h=2r+e at cols 512r+256e. Two exp instructions halve the
            # ACT latency chain; P col order m = (r, e).
            P = ppool.tile([128, 2048], BF16, tag="P", name="P")
            S = s_ps.tile([128, 2048], F32, tag="S", name="S")
            for e in range(2):
                for r in range(4):
                    if e == 0:
                        nc.tensor.matmul(
                            out=S[:, 512 * r:512 * r + 256],
                            lhsT=qg[32 * r:32 * r + 16],
                            rhs=k_ap[32 * r:32 * r + 16],
                            tile_position=(32 * r, 0))
                    else:
                        nc.tensor.matmul(
                            out=S[:, 512 * r + 256:512 * r + 512],
                            lhsT=qzg[32 * r:32 * r + 32],
                            rhs=k_ap[32 * r:32 * r + 32],
                            tile_position=(32 * r, 0))
            nc.scalar.activation(out=P, in_=S,
                                 func=mybir.ActivationFunctionType.Exp)

            # v transpose (tokens x channels)
            vT = vt_ps.tile([128, 128], BF16, tag="vT", name="vT")
            nc.tensor.transpose(vT, vg, ident)

            # row sums r[q, m], m = 2r + e for head 2r+e: GPSIMD L1 + DVE tree
            Pr = P.rearrange("c (m k) -> c m k", m=8)
            scr = small.tile([128, 8, 128], BF16, tag="scr", name="scr")
            nc.gpsimd.tensor_add(out=scr, in0=Pr[:, :, 0:128], in1=Pr[:, :, 128:256])
            nc.vector.tensor_add(out=scr[:, :, 0:64], in0=scr[:, :, 0:64], in1=scr[:, :, 64:128])
            nc.vector.tensor_add(out=scr[:, :, 0:32], in0=scr[:, :, 0:32], in1=scr[:, :, 32:64])
            rs = small.tile([128, 8], F32, tag="rs", name="rs")
            nc.vector.reduce_sum(rs, scr[:, :, 0:32], axis=mybir.AxisListType.X)
            rinv = small.tile([128, 8], F32, tag="rinv", name="rinv")
            nc.vector.reciprocal(rinv, rs)
            rinvb = small.tile([128, 8], BF16, tag="rinvb", name="rinvb")
            nc.vector.tensor_copy(rinvb, rinv)

            # v' = vT * (1/r): channel (j,e,d) uses rinv[m=4e+j]
            vs = small.tile([128, 128], BF16, tag="vs", name="vs")
            rinv_bcast = bass.AP(
                tensor=rinvb.tensor, offset=rinvb.offset,
                ap=[rinvb.ap[0], [2, 4], [1, 2], [0, 16]])
            nc.vector.tensor_tensor(out=vs, in0=vT, in1=rinv_bcast,
                                    op=mybir.AluOpType.mult)
            # zero-padded v' for odd heads (psum out base must be 32-aligned)
            vz = small.tile([128, 128], BF16, tag="vz", name="vz")
            nc.gpsimd.memset(vz, 0.0)
            vzv = vz.rearrange("c (j e d) -> c j e d", j=4, e=2)
            vTv = vT.rearrange("c (j e d) -> c j e d", j=4, e=2)
            rinv_odd = bass.AP(
                tensor=rinvb.tensor, offset=rinvb.offset + 1,
                ap=[rinvb.ap[0], [2, 4], [0, 16]])
            nc.vector.tensor_tensor(out=vzv[:, :, 1], in0=vTv[:, :, 1],
                                    in1=rinv_odd, op=mybir.AluOpType.mult)

            # AV col-tiled: head 2j+e -> psum rows 32j+16e+d
            # odd head first (M=32 zero-padded, start=True), then even accumulates
            av = av_ps.tile([128, 256], F32, tag="av", name="av")
            for j in range(4):
                nc.tensor.matmul(
                    out=av[32 * j:32 * j + 32, :],
                    lhsT=vz[:, 32 * j:32 * j + 32],
                    rhs=P[:, 512 * j + 256:512 * j + 512],
                    tile_position=(0, 32 * j),
                    start=True, stop=False, skip_group_check=True)
                nc.tensor.matmul(
                    out=av[32 * j:32 * j + 16, :],
                    lhsT=vs[:, 32 * j:32 * j + 16],
                    rhs=P[:, 512 * j:512 * j + 256],
                    tile_position=(0, 32 * j),
                    start=False, stop=True, skip_group_check=True)
            if g % 2 == 0:
                nc.vector.tensor_copy(out=av_out, in_=av)
            else:
                nc.scalar.activation(out=av_out, in_=av,
                                     func=mybir.ActivationFunctionType.Copy)

        def final_tiles(st, img, sc):
            Fs = st["F"]
            for oc in range(2):
                outv = out_d[img, 128 * oc:128 * oc + 128].rearrange("c x y -> c (x y)")
                pt = dense_ps.tile([128, 512], F32, tag="dps", name="dps")
                for cc in range(2):
                    nc.tensor.matmul(out=pt, lhsT=wr_sb[cc][oc],
                                     rhs=Fs[cc][:, 512 * sc:512 * sc + 512],
                                     start=(cc == 0), stop=False,
                                     skip_group_check=True)
                for br, avsb in ((0, st["AVH"]), (1, st["AVV"])):
                    nc.tensor.matmul(out=pt, lhsT=wproj_sb[br][oc],
                                     rhs=avsb[:, 512 * sc:512 * sc + 512],
                                     start=False, stop=(br == 1),
                                     skip_group_check=True)
                osb = small.tile([128, 512], F32, tag="osb", name="osb")
                nc.scalar.activation(out=osb, in_=pt,
                                     func=mybir.ActivationFunctionType.Copy)
                nc.sync.dma_start(outv[:, 512 * sc:512 * sc + 512], osb)

        # ---- main schedule: image-0 dense up front; then per image attention
        # with next image's dense work and this image's final interleaved ----
        states = [dict() for _ in range(B_PER_CORE)]
        gen = dense_gen(0, states[0], h_first=True)
        for _ in gen:
            pass
        for img in range(B_PER_CORE):
            st = states[img]
            nxt = dense_gen(img + 1, states[img + 1]) if img + 1 < B_PER_CORE else None
            st["AVH"] = avpool.tile([128, 4096], BF16, tag="avH", name="avH")
            st["AVV"] = avpool1.tile([128, 4096], BF16, tag="avV", name="avV")
            for br in range(2):
                for g in range(16):
                    attention_group(st, br, g)
                    if nxt is not None:
                        for _ in range(2):
                            if next(nxt, None) is None:
                                nxt = None
                                break
                    if br == 1 and g % 2 == 1:
                        final_tiles(st, img, g // 2)
            while nxt is not None and next(nxt, None) is not None:
                pass


def build_kernel():
    """Build + schedule the per-core SPMD program (2 images per core)."""
    nc = bacc.Bacc("TRN2", target_bir_lowering=False, debug=False,
                   enable_asserts=False)

    fmap_d = nc.dram_tensor("fmap", (B_PER_CORE, 256, X, Y), F32R, kind="ExternalInput").ap()
    out_d = nc.dram_tensor("out", (B_PER_CORE, 256, X, Y), F32, kind="ExternalOutput").ap()
    wall_d = nc.dram_tensor("wall", (128, 2560), F32R, kind="ExternalInput").ap()
    wpall_d = nc.dram_tensor("wpall", (128, 512), BF16, kind="ExternalInput").ap()

    with tile.TileContext(nc) as tc:
        emit_kernel(tc, fmap_d, out_d, wall_d, wpall_d)

    nc.compile()
    return nc


_NC_CACHE = None


def _get_nc():
    global _NC_CACHE
    if _NC_CACHE is None:
        _NC_CACHE = build_kernel()
    return _NC_CACHE


def kernel(fmap, W_qvh, W_qvv, W_k, W_lepe, W_proj):
    fmap = np.ascontiguousarray(np.asarray(fmap, dtype=np.float32))
    w = prep_weights(W_qvh, W_qvv, W_k, W_lepe, W_proj)
    nc = _get_nc()
    in_maps = []
    for i in range(N_CORES):
        in_maps.append({
            "fmap": np.ascontiguousarray(fmap[B_PER_CORE * i:B_PER_CORE * (i + 1)]),
            "wall": w["wall"], "wpall": w["wpall"],
        })
    res = run_bass_kernel_spmd(nc, in_maps, core_ids=list(range(N_CORES)))
    out = np.concatenate([res.results[i]["out"] for i in range(N_CORES)], axis=0)
    return out


# revision 20
# speedup vs baseline: 1.3876x; 1.3876x over previous
"""CSWin strip-window attention + pooling kernel for Trainium2 (8 NeuronCores).

Problem: nn_CswinPool — fmap (16, 256, 64, 64) f32.
  qv_h = conv(fmap[:, :128], W_qvh, stride (2,1));  qv_v = conv(fmap[:, 128:], W_qvv, stride (1,2))
  k_all = W_k @ fmap; lepe_all = W_lepe @ fmap (1x1 convs)
  two strip-window attention branches (horizontal / vertical), 8 heads, d=16
  out = fmap + W_proj @ (attn_out + lepe_all)

Strategy (data-parallel over batch, 2 images per core):
  - lepe folded host-side: out = W_r @ F + W_projH' @ AV_H + W_projV' @ AV_V,
    W_r = I + W_proj @ W_lepe  (identity residual folded in)
  - channel permutation sigma puts head h at partitions 32*(h//2)+16*(h%2)
    so row/col-tiled PE matmuls hit quadrant-aligned SBUF bases
  - dense 1x1/strided convs as fp32r matmuls (full PE rate, no conversion pass)
  - QK row-tiled 4 concurrent heads (even heads K=16; odd heads K=32 with a
    zero-padded q buffer produced by an extra zeroed-weight conv output)
  - softmax without max-subtraction (|S| < 0.6 for this problem), exp on ACT
    in one (128, 2048) instruction per group, row-sums via GPSIMD+DVE tree
  - AV col-tiled, P moving, v'(= v/r) stationary; output (h,d)-contiguous in
    PSUM so evac is full-128-partition
"""
import sys
import os

for _p in ("/opt/trn_rl_repo", "/root/.axon_site/_ro/trn_rl_repo"):
    if _p not in sys.path and os.path.isdir(_p):
        sys.path.insert(0, _p)

import numpy as np
import ml_dtypes
from contextlib import ExitStack

import concourse.bass as bass
import concourse.tile as tile
from concourse import bacc, mybir
from concourse.bass_utils import run_bass_kernel_spmd
from concourse.masks import make_identity

F32 = mybir.dt.float32
F32R = mybir.dt.float32r
BF16 = mybir.dt.bfloat16

N_CORES = 8
B_PER_CORE = 2
DIM = 256
HEADS = 8
D = 16           # per-branch head dim
X = Y = 64
S_SPATIAL = X * Y
SCALE = (DIM // HEADS) ** -0.5

# partition p holds reference channel SIGMA[p]
SIGMA = np.zeros(128, dtype=np.int64)
for _h in range(8):
    for _d in range(16):
        SIGMA[32 * (_h // 2) + 16 * (_h % 2) + _d] = _h * 16 + _d


def prep_weights(W_qvh, W_qvv, W_k, W_lepe, W_proj):
    """Host-side weight preparation: permutations, folds, lhsT layouts."""
    W_qvh = np.asarray(W_qvh, dtype=np.float32)
    W_qvv = np.asarray(W_qvv, dtype=np.float32)
    W_k = np.asarray(W_k, dtype=np.float32)
    W_lepe = np.asarray(W_lepe, dtype=np.float32)
    W_proj = np.asarray(W_proj, dtype=np.float32)

    # wk[half, cc] : (128 c, 128 o) lhsT for k matmul, output channels sigma-permuted
    wk = np.zeros((2, 2, 128, 128), dtype=np.float32)
    for half in range(2):
        Wh = W_k[128 * half:128 * half + 128][SIGMA]      # (128 o_perm, 256 c)
        for cc in range(2):
            wk[half, cc] = Wh[:, 128 * cc:128 * cc + 128].T

    # wcv[branch, kind(q=0,v=1,qz=2), tap] : (128 c, 128 o)
    wcv = np.zeros((2, 3, 2, 128, 128), dtype=np.float32)
    zero_even = np.ones(128, dtype=np.float32)
    zero_even[(np.arange(128) % 32) < 16] = 0.0            # keep only odd-slot channels
    for br, Wc in ((0, W_qvh[:, :, :, 0]), (1, W_qvv[:, :, 0, :])):
        # Wc: (256 o, 128 c, 2 tap)
        Wq = Wc[:128][SIGMA] * SCALE                       # (128 o_perm, 128 c, 2)
        Wv = Wc[128:][SIGMA]
        Wqz = Wq * zero_even[:, None, None]
        for t in range(2):
            wcv[br, 0, t] = Wq[:, :, t].T
            wcv[br, 1, t] = Wv[:, :, t].T
            wcv[br, 2, t] = Wqz[:, :, t].T

    # wr[cc, oc] : (128 c, 128 o) lhsT of W_r = I + W_proj @ W_lepe
    W_r = np.eye(256, dtype=np.float32) + W_proj @ W_lepe
    wr = np.zeros((2, 2, 128, 128), dtype=np.float32)
    for cc in range(2):
        for oc in range(2):
            wr[cc, oc] = W_r[128 * oc:128 * oc + 128, 128 * cc:128 * cc + 128].T

    # wproj[branch, oc] : (128 c', 128 o) bf16, c' in sigma layout
    wproj = np.zeros((2, 2, 128, 128), dtype=ml_dtypes.bfloat16)
    for br in range(2):
        Wp = W_proj[:, 128 * br:128 * br + 128][:, SIGMA]  # (256 o, 128 c'_perm)
        for oc in range(2):
            wproj[br, oc] = Wp[128 * oc:128 * oc + 128, :].T.astype(ml_dtypes.bfloat16)

    wall = np.zeros((128, 20 * 128), dtype=np.float32)
    i = 0
    for half in range(2):
        for cc in range(2):
            wall[:, 128 * i:128 * i + 128] = wk[half, cc]
            i += 1
    for br in range(2):
        for kind in range(3):
            for t in range(2):
                wall[:, 128 * i:128 * i + 128] = wcv[br, kind, t]
                i += 1
    for cc in range(2):
        for oc in range(2):
            wall[:, 128 * i:128 * i + 128] = wr[cc, oc]
            i += 1
    wpall = np.zeros((128, 2 * 2 * 128), dtype=ml_dtypes.bfloat16)
    i = 0
    for br in range(2):
        for oc in range(2):
            wpall[:, 128 * i:128 * i + 128] = wproj[br, oc]
            i += 1
    return {"wall": wall, "wpall": wpall}


def r32(ap):
    return ap.bitcast(F32R)


def emit_kernel(tc, fmap_d, out_d, wall_d, wpall_d):
    nc = tc.nc
    with ExitStack() as ctx:
        singles = ctx.enter_context(tc.tile_pool(name="singles", bufs=1))
        fpool = ctx.enter_context(tc.tile_pool(name="fpool", bufs=2))
        kpool = ctx.enter_context(tc.tile_pool(name="kpool", bufs=1))
        kpool2 = ctx.enter_context(tc.tile_pool(name="kpool2", bufs=2))
        qvpool = ctx.enter_context(tc.tile_pool(name="qvpool", bufs=1))
        qvpool2 = ctx.enter_context(tc.tile_pool(name="qvpool2", bufs=2))
        avpool = ctx.enter_context(tc.tile_pool(name="avpool", bufs=2))
        avpool1 = ctx.enter_context(tc.tile_pool(name="avpool1", bufs=1))
        ppool = ctx.enter_context(tc.tile_pool(name="ppool", bufs=3))
        small = ctx.enter_context(tc.tile_pool(name="small", bufs=4))
        dense_ps = ctx.enter_context(tc.tile_pool(name="dense_ps", bufs=2, space="PSUM"))
        s_ps = ctx.enter_context(tc.tile_pool(name="s_ps", bufs=2, space="PSUM"))
        vt_ps = ctx.enter_context(tc.tile_pool(name="vt_ps", bufs=1, space="PSUM"))
        av_ps = ctx.enter_context(tc.tile_pool(name="av_ps", bufs=1, space="PSUM"))

        # ---- weights (two big DMAs; slices as lhsT views) ----
        wall_sb = singles.tile([128, 2560], F32R, tag="wall", name="wall")
        nc.sync.dma_start(wall_sb, wall_d)
        wpall_sb = singles.tile([128, 512], BF16, tag="wpall", name="wpall")
        nc.sync.dma_start(wpall_sb, wpall_d)

        def wslice(i):
            return wall_sb[:, 128 * i:128 * i + 128]

        wk_sb = [[wslice(h * 2 + c) for c in range(2)] for h in range(2)]
        wcv_sb = [[[wslice(4 + b * 6 + k * 2 + t) for t in range(2)]
                   for k in range(3)] for b in range(2)]
        wr_sb = [[wslice(16 + c * 2 + o) for o in range(2)] for c in range(2)]
        wproj_sb = [[wpall_sb[:, 128 * (b * 2 + o):128 * (b * 2 + o) + 128] for o in range(2)] for b in range(2)]
        ident = singles.tile([128, 128], BF16, tag="ident", name="ident")
        make_identity(nc, ident)

        def dense_gen(img, st, h_first=False):
            """Yield after each F-chunk DMA / dense-conv chunk emission.

            Order matters: items whose destination buffers are free
            (double-buffered) come first so interleaved emission into the
            previous image's attention doesn't stall the pipeline."""
            F0 = fpool.tile([128, 4096], F32R, tag="F0", name="F0")
            F1 = fpool.tile([128, 4096], F32R, tag="F1", name="F1")
            st["F"] = (F0, F1)
            fm = [fmap_d[img, 0:128].rearrange("c x y -> c (x y)"),
                  fmap_d[img, 128:256].rearrange("c x y -> c (x y)")]
            for half, Ft in ((0, F0), (1, F1)):
                for n in range(8):
                    nc.sync.dma_start(Ft[:, 512 * n:512 * n + 512],
                                      fm[half][:, 512 * n:512 * n + 512])
                    yield
            Fs = (F0, F1)

            def k_chunks(half, ksb):
                for n in range(8):
                    pt = dense_ps.tile([128, 512], F32, tag="dps", name="dps")
                    for cc in range(2):
                        nc.tensor.matmul(out=pt, lhsT=wk_sb[half][cc],
                                         rhs=Fs[cc][:, 512 * n:512 * n + 512],
                                         start=(cc == 0), stop=(cc == 1))
                    if n % 4 == 3:
                        nc.scalar.activation(out=ksb[:, 512 * n:512 * n + 512], in_=pt,
                                             func=mybir.ActivationFunctionType.Copy)
                    else:
                        nc.vector.tensor_copy(out=ksb[:, 512 * n:512 * n + 512], in_=pt)
                    yield

            def qv_chunks(br, kind, dst, reorder):
                Fv = (Fs[0].rearrange("c (i t y) -> c i t y", t=2, y=64) if br == 0
                      else Fs[1].rearrange("c (x j t) -> c x j t", j=32, t=2))
                dstv = dst.rearrange("c (g i p) -> c i g p", g=16, p=4) if reorder else dst
                for n in range(4):
                    pt = dense_ps.tile([128, 512], F32, tag="dps", name="dps")
                    for t in range(2):
                        rhs = (Fv[:, 8 * n:8 * n + 8, t, :] if br == 0
                               else Fv[:, 16 * n:16 * n + 16, :, t])
                        nc.tensor.matmul(out=pt, lhsT=wcv_sb[br][kind][t], rhs=rhs,
                                         start=(t == 0), stop=(t == 1))
                    if reorder:
                        nc.vector.tensor_copy(out=dstv[:, 8 * n:8 * n + 8], in_=pt)
                    else:
                        nc.vector.tensor_copy(out=dst[:, 512 * n:512 * n + 512], in_=pt)
                    yield

            def emit_V():
                kV = kpool2.tile([128, 4096], BF16, tag="kV", name="kV")
                st["kV"] = kV
                yield from k_chunks(1, kV)
                qV = qvpool2.tile([128, 2048], BF16, tag="qV", name="qV")
                st["qV"] = qV
                yield from qv_chunks(1, 0, qV, False)
                vV = qvpool2.tile([128, 2048], BF16, tag="vV", name="vV")
                st["vV"] = vV
                yield from qv_chunks(1, 1, vV, False)

            def emit_H():
                kH = kpool.tile([128, 4096], BF16, tag="kH", name="kH")
                st["kH"] = kH
                yield from k_chunks(0, kH)
                qH = qvpool.tile([128, 2048], BF16, tag="qH", name="qH")
                st["qH"] = qH
                yield from qv_chunks(0, 0, qH, True)
                vH = qvpool.tile([128, 2048], BF16, tag="vH", name="vH")
                st["vH"] = vH
                yield from qv_chunks(0, 1, vH, True)
                qzH = qvpool2.tile([128, 2048], BF16, tag="qz", name="qz", bufs=3)
                st["qzH"] = qzH
                yield from qv_chunks(0, 2, qzH, True)

            # for image 0 (nothing to overlap) emit H-branch buffers first so
            # attention starts ASAP; for interleaved images emit the
            # double-buffered (no-WAR-wait) V-branch destinations first.
            if h_first:
                yield from emit_H()
                yield from emit_V()
            else:
                yield from emit_V()
                yield from emit_H()
            qzV = qvpool2.tile([128, 2048], BF16, tag="qz", name="qz", bufs=3)
            st["qzV"] = qzV
            yield from qv_chunks(1, 2, qzV, False)

        def attention_group(st, br, g):
            ksb = st["kH"] if br == 0 else st["kV"]
            qsb = st["qH"] if br == 0 else st["qV"]
            vsb = st["vH"] if br == 0 else st["vV"]
            qzsb = st["qzH"] if br == 0 else st["qzV"]
            avsb = st["AVH"] if br == 0 else st["AVV"]
            kv = ksb.rearrange("c (x y) -> c x y", y=64)
            avv = avsb.rearrange("c (x y) -> c x y", y=64)
            if br == 0:
                k_ap = kv[:, :, 4 * g:4 * g + 4]                # (128, 64, 4)
                av_out = avv[:, :, 4 * g:4 * g + 4]
            else:
                k_ap = kv[:, 4 * g:4 * g + 4, :]                # (128, 4, 64)
                av_out = avv[:, 4 * g:4 * g + 4, :]
            qg = qsb[:, 128 * g:128 * g + 128]                  # grouped tokens
            qzg = qzsb[:, 128 * g:128 * g + 128]
            vg = vsb[:, 128 * g:128 * g + 128]

            # QK in two half-groups by ROW-TILE PAIRS: half hb covers tiles
            # {2hb, 2hb+1}; within a half, tile r owns one full bank (cols
            # 512*(r-2hb)+256e) so no two concurrent matmuls share a PSUM
            # bank. P cols stay 512r+256e globally (m = 2r+e unchanged).
            P = ppool.tile([128, 2048], BF16, tag="P", name="P")
            for hb in range(2):
                Sh = s_ps.tile([128, 1024], F32, tag="S", name="S")
                for r in (2 * hb, 2 * hb + 1):
                    c0 = 512 * (r - 2 * hb)
                    nc.tensor.matmul(
                        out=Sh[:, c0:c0 + 256],
                        lhsT=qg[32 * r:32 * r + 16],
                        rhs=k_ap[32 * r:32 * r + 16],
                        tile_position=(32 * r, 0))
                    nc.tensor.matmul(
                        out=Sh[:, c0 + 256:c0 + 512],
                        lhsT=qzg[32 * r:32 * r + 32],
                        rhs=k_ap[32 * r:32 * r + 32],
                        tile_position=(32 * r, 0))
                nc.scalar.activation(out=P[:, 1024 * hb:1024 * hb + 1024], in_=Sh,
                                     func=mybir.ActivationFunctionType.Exp)

            # v transpose (tokens x channels)
            vT = vt_ps.tile([128, 128], BF16, tag="vT", name="vT")
            nc.tensor.transpose(vT, vg, ident)

            # row sums r[q, m], m = 2r + e for head 2r+e: GPSIMD L1 + DVE tree
            Pr = P.rearrange("c (m k) -> c m k", m=8)
            scr = small.tile([128, 8, 128], BF16, tag="scr", name="scr")
            nc.gpsimd.tensor_add(out=scr, in0=Pr[:, :, 0:128], in1=Pr[:, :, 128:256])
            nc.vector.tensor_add(out=scr[:, :, 0:64], in0=scr[:, :, 0:64], in1=scr[:, :, 64:128])
            nc.vector.tensor_add(out=scr[:, :, 0:32], in0=scr[:, :, 0:32], in1=scr[:, :, 32:64])
            rs = small.tile([128, 8], F32, tag="rs", name="rs")
            nc.vector.reduce_sum(rs, scr[:, :, 0:32], axis=mybir.AxisListType.X)
            rinv = small.tile([128, 8], F32, tag="rinv", name="rinv")
            nc.vector.reciprocal(rinv, rs)
            rinvb = small.tile([128, 8], BF16, tag="rinvb", name="rinvb")
            nc.vector.tensor_copy(rinvb, rinv)

            # v' = vT * (1/r): channel (j,e,d) uses rinv[m=4e+j]
            vs = small.tile([128, 128], BF16, tag="vs", name="vs")
            rinv_bcast = bass.AP(
                tensor=rinvb.tensor, offset=rinvb.offset,
                ap=[rinvb.ap[0], [2, 4], [1, 2], [0, 16]])
            nc.vector.tensor_tensor(out=vs, in0=vT, in1=rinv_bcast,
                                    op=mybir.AluOpType.mult)
            # zero-padded v' for odd heads (psum out base must be 32-aligned)
            vz = small.tile([128, 128], BF16, tag="vz", name="vz")
            nc.gpsimd.memset(vz, 0.0)
            vzv = vz.rearrange("c (j e d) -> c j e d", j=4, e=2)
            vTv = vT.rearrange("c (j e d) -> c j e d", j=4, e=2)
            rinv_odd = bass.AP(
                tensor=rinvb.tensor, offset=rinvb.offset + 1,
                ap=[rinvb.ap[0], [2, 4], [0, 16]])
            nc.vector.tensor_tensor(out=vzv[:, :, 1], in0=vTv[:, :, 1],
                                    in1=rinv_odd, op=mybir.AluOpType.mult)

            # AV col-tiled: head 2j+e -> psum rows 32j+16e+d
            # odd head first (M=32 zero-padded, start=True), then even accumulates
            av = av_ps.tile([128, 256], F32, tag="av", name="av")
            for j in range(4):
                nc.tensor.matmul(
                    out=av[32 * j:32 * j + 32, :],
                    lhsT=vz[:, 32 * j:32 * j + 32],
                    rhs=P[:, 512 * j + 256:512 * j + 512],
                    tile_position=(0, 32 * j),
                    start=True, stop=False, skip_group_check=True)
                nc.tensor.matmul(
                    out=av[32 * j:32 * j + 16, :],
                    lhsT=vs[:, 32 * j:32 * j + 16],
                    rhs=P[:, 512 * j:512 * j + 256],
                    tile_position=(0, 32 * j),
                    start=False, stop=True, skip_group_check=True)
            if g % 2 == 0:
                nc.vector.tensor_copy(out=av_out, in_=av)
            else:
                nc.scalar.activation(out=av_out, in_=av,
                                     func=mybir.ActivationFunctionType.Copy)

        def final_tiles(st, img, sc):
            Fs = st["F"]
            for oc in range(2):
                outv = out_d[img, 128 * oc:128 * oc + 128].rearrange("c x y -> c (x y)")
                pt = dense_ps.tile([128, 512], F32, tag="dps", name="dps")
                for cc in range(2):
                    nc.tensor.matmul(out=pt, lhsT=wr_sb[cc][oc],
                                     rhs=Fs[cc][:, 512 * sc:512 * sc + 512],
                                     start=(cc == 0), stop=False,
                                     skip_group_check=True)
                for br, avsb in ((0, st["AVH"]), (1, st["AVV"])):
                    nc.tensor.matmul(out=pt, lhsT=wproj_sb[br][oc],
                                     rhs=avsb[:, 512 * sc:512 * sc + 512],
                                     start=False, stop=(br == 1),
                                     skip_group_check=True)
                osb = small.tile([128, 512], F32, tag="osb", name="osb")
                nc.scalar.activation(out=osb, in_=pt,
                                     func=mybir.ActivationFunctionType.Copy)
                nc.sync.dma_start(outv[:, 512 * sc:512 * sc + 512], osb)

        # ---- main schedule: image-0 dense up front; then per image attention
        # with next image's dense work and this image's final interleaved ----
        states = [dict() for _ in range(B_PER_CORE)]
        gen = dense_gen(0, states[0], h_first=True)
        for _ in gen:
            pass
        for img in range(B_PER_CORE):
            st = states[img]
            nxt = dense_gen(img + 1, states[img + 1]) if img + 1 < B_PER_CORE else None
            st["AVH"] = avpool.tile([128, 4096], BF16, tag="avH", name="avH")
            st["AVV"] = avpool1.tile([128, 4096], BF16, tag="avV", name="avV")
            for br in range(2):
                for g in range(16):
                    attention_group(st, br, g)
                    if nxt is not None:
                        for _ in range(2):
                            if next(nxt, None) is None:
                                nxt = None
                                break
                    if br == 1 and g % 2 == 1:
                        final_tiles(st, img, g // 2)
            while nxt is not None and next(nxt, None) is not None:
                pass


def build_kernel():
    """Build + schedule the per-core SPMD program (2 images per core)."""
    nc = bacc.Bacc("TRN2", target_bir_lowering=False, debug=False,
                   enable_asserts=False)

    fmap_d = nc.dram_tensor("fmap", (B_PER_CORE, 256, X, Y), F32R, kind="ExternalInput").ap()
    out_d = nc.dram_tensor("out", (B_PER_CORE, 256, X, Y), F32, kind="ExternalOutput").ap()
    wall_d = nc.dram_tensor("wall", (128, 2560), F32R, kind="ExternalInput").ap()
    wpall_d = nc.dram_tensor("wpall", (128, 512), BF16, kind="ExternalInput").ap()

    with tile.TileContext(nc) as tc:
        emit_kernel(tc, fmap_d, out_d, wall_d, wpall_d)

    nc.compile()
    return nc


_NC_CACHE = None


def _get_nc():
    global _NC_CACHE
    if _NC_CACHE is None:
        _NC_CACHE = build_kernel()
    return _NC_CACHE


def kernel(fmap, W_qvh, W_qvv, W_k, W_lepe, W_proj):
    fmap = np.ascontiguousarray(np.asarray(fmap, dtype=np.float32))
    w = prep_weights(W_qvh, W_qvv, W_k, W_lepe, W_proj)
    nc = _get_nc()
    in_maps = []
    for i in range(N_CORES):
        in_maps.append({
            "fmap": np.ascontiguousarray(fmap[B_PER_CORE * i:B_PER_CORE * (i + 1)]),
            "wall": w["wall"], "wpall": w["wpall"],
        })
    res = run_bass_kernel_spmd(nc, in_maps, core_ids=list(range(N_CORES)))
    out = np.concatenate([res.results[i]["out"] for i in range(N_CORES)], axis=0)
    return out
